# revision 19
# baseline (speedup 1.0000x reference)
"""Trainium2 Bass kernel for AttentionMLP (nn_AttentionMLP_72997264163220).

Reference computation:
  k/q/v = x @ W{k,q,v}.T + b      (D=3800 -> D)
  scores = q @ k.T / sqrt(D); attn = softmax(scores, -1)
  attended = attn @ v; h = attended.mean(seq)
  h = sigmoid(h @ W1.T + b1); h = sigmoid(h @ W2.T + b2); out = h @ W3.T + b3

Algebraic simplifications (all weight-only, folded on the host):
  1. scores = x' M x'^T with M = Wq'^T Wk' / sqrt(D)  (bias via a unit
     feature at d=3800), so q/k are never materialized.
  2. The mean over the sequence commutes with attention and the linear
     V/W1 layers:  z1 = W1(Wv(abar @ x') + bv) + b1 = Wf @ (abar @ x')
     with Wf = W1 @ Wv (+ bias col).  On device this is evaluated as
       Gt = x' @ Wf^T   [tokens, H]   (reuses the x tiles already in
                                       SBUF for the scores matmuls)
       z1 = Gt^T @ abar [H, batch]
     so neither v, the attended tensor, xa, nor Wv/W1 are ever needed:
     only the 512xD folded Wf (fp8, ~2MB vs ~19MB for Wv+W1).

Sharding: data-parallel over batch. 16 batches -> 8 cores x 2 batches
(512 tokens per core). All weights replicated, host pre-transposed /
tiled / cast. Big matmuls in fp8 DoubleRow (fp32 PSUM accumulate);
softmax in fp32, MLP tail in bf16.

DMA queues: m8 (14.7MB, the big stream) rides the Sync queue alone;
x8 on the Vector queue; Wf/W2/W3/e0b on the Scalar queue. This keeps
the critical m8 stream free of head-of-line blocking (the previous
version put ~39MB on one queue and spent ~30us before compute ramped).

Device dataflow per core (SBUF partition dim always first; D padded to
3840 = 30*128 with a bias feature at d=3800):
  xT    [128, 30, 512] fp8  x^T (dp, kc, token); row d=3800 == 1
  per e-tile et in 30:  t1[et] = M8[et]^T x8   (DoubleRow pairs)
  scores[2b+it] psum [128,256] += t1_et_slice^T @ x8_slice  over et
  softmax rows (fp32, on ACT/DVE) -> attn bf16 [128(i), 256(j)]
  abar[b] = colsum_i(attn) * (XASCALE/S)  via matmul with a const vec
  Gt[tc]  = x8_tc^T @ Wf^T  (DoubleRow, abar-independent: runs during
            softmax) -> gt_sb [128(tok), 4, 512] bf16
  z1 = Gt^T abar -> sigmoid -> bf16 MLP (biases via unit rows/feature)
"""

import sys
import types

import numpy as np

if "/opt/trn_rl_repo" not in sys.path:
    sys.path.insert(0, "/opt/trn_rl_repo")


# ---------------------------------------------------------------------------
# NTFF profile hook shim (antenv.axon_hooks is absent in this image). Needed
# only when profiling (trace=True); harmless otherwise.
# ---------------------------------------------------------------------------
def _install_ntff_hook():
    try:
        import antenv  # noqa: F401

        if "antenv.axon_hooks" in sys.modules:
            return
        hooks_mod = types.ModuleType("antenv.axon_hooks")
        hooks_mod._hook = None

        def set_axon_ntff_profile_hook(h):
            hooks_mod._hook = h

        def get_axon_ntff_profile_hook():
            return hooks_mod._hook

        hooks_mod.set_axon_ntff_profile_hook = set_axon_ntff_profile_hook
        hooks_mod.get_axon_ntff_profile_hook = get_axon_ntff_profile_hook
        sys.modules["antenv.axon_hooks"] = hooks_mod
        import antenv as _a

        _a.axon_hooks = hooks_mod
        from trn_agent_boot.trn_boot import _ntff_profile_via_ctypes

        set_axon_ntff_profile_hook(
            _ntff_profile_via_ctypes("/opt/axon/libaxon_pjrt.so")
        )
    except Exception:
        pass


_install_ntff_hook()


def _install_verbose_cc_hook():
    """Wrap the PJRT->python compile callback so real tracebacks surface
    instead of an opaque 'CallFunctionObjArgs' error."""
    try:
        import traceback

        from concourse import bass2jax

        bass2jax.install_neuronx_cc_hook()
        import libneuronxla

        if getattr(libneuronxla, "_ant_verbose_wrap", False):
            return
        orig = libneuronxla.neuronx_cc

        def wrapped(*a, **k):
            try:
                return orig(*a, **k)
            except BaseException:
                traceback.print_exc()
                sys.stderr.flush()
                raise

        libneuronxla.neuronx_cc = wrapped
        libneuronxla._ant_verbose_wrap = True
        bass2jax.install_neuronx_cc_hook = lambda: None
    except Exception:
        pass


import bass_rust
import ml_dtypes
import concourse.bass as bass
import concourse.tile as tile
from concourse import mybir
from concourse.bass_utils import run_bass_kernel_spmd
from concourse.vector_clock import ScopedClock

BF16 = ml_dtypes.bfloat16

N_CORES = 8
B = 16  # batches total
S = 256  # seq len
D = 3800  # feature dim
H = 512  # hidden
C = 10  # classes

BLOC = B // N_CORES  # batches per core = 2
T = BLOC * S  # tokens per core = 512
DP = 3840  # D padded (+1 bias feature, up to 30*128)
KC = DP // 128  # 30 contraction chunks
ET = DP // 128  # 30 e-tiles of 128
PAIRS = KC // 2  # 15 DoubleRow chunk pairs
F32 = mybir.dt.float32
BF = mybir.dt.bfloat16
F8 = mybir.dt.float8e4
F8NP = mybir.dt.np(F8)  # ml_dtypes.float8_e4m3
# fp8 scale factors: weights are ~U(+-1/sqrt(3800)) which lands in e4m3's
# subnormal range, so weights are scaled up and the product scales are
# folded back out downstream (softmax scale / sigmoid scale).
SC_SCALE = 4096.0  # on M = Wq^T Wk / sqrt(D); scores' = 4096 * scores
XASCALE = 16.0  # on abar (via the ones vector)
WF_SCALE = 256.0  # on Wf = W1 @ Wv; z1' = XASCALE*WF_SCALE * z1


class SplitDrainTileContext(tile.TileContext):
    """This walrus build rejects >1 sync-wait on the tail Drain; split the
    global-clock waits across a chain of single-wait drain instructions."""

    MAXW = 1

    def _drain_and_barrier(self, tick_clock, wait_clock):
        nc = self.nc
        drain_inst = nc.sync.drain()
        wait_clock.add_sem_waits(
            drain_inst.ins, ScopedClock({None: tick_clock.global_clock})
        )
        si = drain_inst.ins.sync_info
        if si is not None and si.on_wait and len(si.on_wait) > self.MAXW:
            waits = list(si.on_wait)
            si.on_wait = waits[: self.MAXW]
            rest = waits[self.MAXW :]
            for i in range(0, len(rest), self.MAXW):
                extra = nc.sync.drain()
                extra.ins.sync_info = bass_rust.SyncInfo(
                    on_wait=rest[i : i + self.MAXW], on_update=[]
                )
        nc.all_engine_barrier()
        assert self.sems is not None
        popped = nc._tile_sem_poison_stack.pop()
        assert popped is self._sem_poison
        nc.clear_and_free_semaphores(list(self.sems.allocated().values()))
        nc.all_engine_barrier()


def _fix_excess_waits(nc, aux_sem, maxw=1):
    """Walrus in this image rejects instructions with more than ~1 sync
    wait. Compute-engine instructions: hoist extra waits onto same-engine
    no-ops inserted just before (sequencers execute in order). DMACopy:
    its waits live in the DGE queue descriptor, so an SP-side chain waits
    on all the original conditions, bumps `aux_sem`, and the descriptor
    waits on aux_sem alone."""
    aux_count = 0
    for f in nc.m.functions:
        for bb in f.blocks:
            insts = bb.instructions
            if not any(
                i.sync_info and i.sync_info.on_wait
                and len(i.sync_info.on_wait) > maxw
                for i in insts
            ):
                continue
            out = []
            for ins in insts:
                si = ins.sync_info
                nw = len(si.on_wait) if si and si.on_wait else 0
                if nw > maxw:
                    waits = list(si.on_wait)
                    if isinstance(ins, mybir.InstDMACopy):
                        for j, w in enumerate(waits):
                            nop = mybir.InstNoOp(name=f"{ins.name}-dw{j}")
                            nop.engine = mybir.EngineType.SP
                            nop.sync_info = bass_rust.SyncInfo(
                                on_wait=[w], on_update=[]
                            )
                            out.append(nop)
                        aux_count += 1
                        inc = mybir.InstNoOp(name=f"{ins.name}-dinc")
                        inc.engine = mybir.EngineType.SP
                        inc.sync_info = bass_rust.SyncInfo(
                            on_wait=[],
                            on_update=[
                                bass_rust.SyncUpdate(
                                    sync_type="semaphore",
                                    id=aux_sem.num,
                                    ant_name=aux_sem.name,
                                    update_mode="sem-add-imm",
                                    update_value=1,
                                    update_reg=None,
                                )
                            ],
                        )
                        out.append(inc)
                        si.on_wait = [
                            bass_rust.SyncWait(
                                sync_type="semaphore",
                                id=aux_sem.num,
                                ant_name=aux_sem.name,
                                wait_mode="sem-ge-imm",
                                wait_value=aux_count,
                                wait_reg=None,
                            )
                        ]
                    else:
                        keep = waits[-maxw:]
                        rest = waits[:-maxw]
                        for j, w in enumerate(rest):
                            nop = mybir.InstNoOp(name=f"{ins.name}-xw{j}")
                            nop.engine = ins.engine
                            nop.sync_info = bass_rust.SyncInfo(
                                on_wait=[w], on_update=[]
                            )
                            out.append(nop)
                        si.on_wait = keep
                out.append(ins)
            bb.instructions = out
    if aux_count:
        # reset aux sem at the very end so a re-executed NEFF starts clean
        f = nc.m.functions[0]
        bb = list(f.blocks)[-1]
        rst = mybir.InstNoOp(name="auxwait-reset")
        rst.engine = mybir.EngineType.SP
        rst.sync_info = bass_rust.SyncInfo(
            on_wait=[],
            on_update=[
                bass_rust.SyncUpdate(
                    sync_type="semaphore",
                    id=aux_sem.num,
                    ant_name=aux_sem.name,
                    update_mode="sem-sub-imm",
                    update_value=aux_count,
                    update_reg=None,
                )
            ],
        )
        il = bb.instructions
        il.append(rst)
        bb.instructions = il


def build_kernel() -> bass.Bass:
    nc = bass.Bass()

    x_d = nc.declare_dram_parameter("x8", [128, PAIRS, 2, T], F8, isOutput=False)
    m8_d = nc.declare_dram_parameter("m8", [ET, 128, PAIRS, 2, 128], F8,
                                     isOutput=False)
    wf_d = nc.declare_dram_parameter("wf", [128, KC, H], F8, isOutput=False)
    # W2/W3 stay fp32: the batch-independent (constant) component of the
    # logits flows through them with no sigmoid small-signal suppression,
    # so bf16 weight error here hits the output directly (~2e-3).
    w2_d = nc.declare_dram_parameter("w2", [128, 5, H], F32, isOutput=False)
    w3_d = nc.declare_dram_parameter("w3", [128, 5, C], F32, isOutput=False)
    e0b_d = nc.declare_dram_parameter("e0b", [128, BLOC], F32, isOutput=False)
    out_d = nc.declare_dram_parameter("outT", [C, BLOC], F32, isOutput=True)

    aux_sem = nc.alloc_semaphore("auxwait")
    with SplitDrainTileContext(nc) as tc:
        with tc.tile_pool(name="persist", bufs=1) as persist:
            _emit(nc, tc, persist, x_d, m8_d, wf_d, w2_d, w3_d, e0b_d, out_d)
    _fix_excess_waits(nc, aux_sem)
    return nc


def _emit(nc, tc, persist, x_d, m8_d, wf_d, w2_d, w3_d, e0b_d, out_d):
    # ------------------ persistent tiles ------------------
    # x8 in 4 group tiles (4+4+4+3 DoubleRow pairs). Grouping matters two
    # ways: Tile dependencies are whole-tile (early matmuls only wait on
    # their own group's DMA), and every DMA costs ~600ns on the shared
    # HWDGE descriptor generator, so per-pair DMAs would make the startup
    # issue-bound (~1.2us/pair was the old kernel's 30us head).
    GRP = [4, 4, 4, 3]
    g0 = [0, 4, 8, 12]
    x8g = [persist.tile([128, GRP[g], 2, T], F8, name=f"x8g{g}", tag=f"x8g{g}")
           for g in range(4)]

    def x8p(p, ts=slice(None)):
        return x8g[p // 4][:, p % 4, :, ts]
    ones_s = persist.tile([128, 1], BF)
    nc.vector.memset(ones_s[:], XASCALE / S)
    a_bar2 = persist.tile([128, 4, BLOC], BF)
    nc.vector.memset(a_bar2[:], 0.0)
    gt_sb = persist.tile([128, 4, H], BF)  # Gt = x' Wf^T  (tok, tc, o)
    # t1 = (M8^T x8): fp8, [d2 within tile, d2-tile, token]
    t1_sb = persist.tile([128, KC, T], F8)
    h1T = persist.tile([128, 5, BLOC], F32)
    h2T = persist.tile([128, 5, BLOC], F32)

    # MLP weights: tiles up-front, DMAs issued a few iterations into
    # phase 1 so they overlap compute instead of the critical startup
    mlpw = tc.alloc_tile_pool(name="mlpw", bufs=1)
    wf_t = mlpw.tile([128, KC, H], F8)
    w2_t = mlpw.tile([128, 5, H], F32)
    w3_t = mlpw.tile([128, 5, C], F32)

    # ---- phase 1a: t1 = M^T x  (scores = x M x^T = t1^T x, M = Wq^T Wk) ----
    DR = mybir.MatmulPerfMode.DoubleRow
    with tc.tile_pool(name="psum_sc", bufs=1, space="PSUM") as psum_sc:
        ps = [
            psum_sc.tile([128, S], F32, name=f"scores{i}", tag=f"scores{i}")
            for i in range(4)  # index = 2*b + it
        ]
        with (
            tc.tile_pool(name="mpool", bufs=1) as mpool,
            tc.tile_pool(name="psum_kq", bufs=1, space="PSUM") as psum_kq,
        ):
            for d2t in range(ET):
                m_t = mpool.tile([128, PAIRS, 2, 128], F8, tag="m8", bufs=3)
                nc.sync.dma_start(m_t[:], m8_d[d2t])
                if d2t == 0:
                    # x8 rides the Activation queue in parallel with the
                    # m8 stream on Sync.
                    for g in range(4):
                        nc.scalar.dma_start(x8g[g][:], x_d[:, g0[g] : g0[g] + GRP[g]])
                if 2 <= d2t < 12 and d2t % 2 == 0:
                    # spread the Wf load over phase-1 iterations, 6 chunks
                    # per DMA (issue cost dominates smaller transfers)
                    kc0 = 3 * (d2t - 2)
                    nc.scalar.dma_start(wf_t[:, kc0 : kc0 + 6, :],
                                        wf_d[:, kc0 : kc0 + 6, :])
                if d2t == 12:
                    nc.scalar.dma_start(w2_t[:], w2_d[:])
                    nc.scalar.dma_start(w3_t[:], w3_d[:])
                    nc.scalar.dma_start(h1T[:, 4, :], e0b_d[:])
                    nc.scalar.dma_start(h2T[:, 4, :], e0b_d[:])

                pt = psum_kq.tile([128, T], F32, tag="pt", bufs=2)
                for p in range(PAIRS):
                    nc.tensor.matmul(
                        pt[:], m_t[:, p], x8p(p),
                        start=(p == 0), stop=(p == PAIRS - 1),
                        perf_mode=DR,
                    )
                nc.vector.tensor_copy(t1_sb[:, d2t, :], pt[:])

            # ---- phase 1b: scores'[i, j] = sum_d2 t1[d2, i] x8[d2, j] ----
            for b in range(BLOC):
                for it in range(2):
                    i0 = b * S + it * 128
                    for p in range(PAIRS):
                        nc.tensor.matmul(
                            ps[2 * b + it][:],
                            t1_sb[:, 2 * p : 2 * p + 2, i0 : i0 + 128],
                            x8p(p, slice(b * S, (b + 1) * S)),
                            start=(p == 0), stop=(p == PAIRS - 1),
                            perf_mode=DR,
                        )

        # ---- phase G: Gt = x' Wf^T (abar-independent; overlaps softmax) ----
        with tc.tile_pool(name="psum_g", bufs=1, space="PSUM") as psum_g:
            for tcn in range(4):
                pg = psum_g.tile([128, H], F32, tag="pg", bufs=2)
                for p in range(PAIRS):
                    nc.tensor.matmul(
                        pg[:],
                        x8p(p, slice(tcn * 128, (tcn + 1) * 128)),
                        wf_t[:, 2 * p : 2 * p + 2, :],
                        start=(p == 0), stop=(p == PAIRS - 1),
                        perf_mode=DR,
                    )
                nc.vector.tensor_copy(gt_sb[:, tcn, :], pg[:])

        # ------------- phase 2: softmax + abar (column means) -------------
        with (
            tc.tile_pool(name="smx", bufs=1) as smx,
            tc.tile_pool(name="psum_ab", bufs=1, space="PSUM") as psum_ab,
        ):
            pab = [
                psum_ab.tile([128, 1], F32, name=f"pab{i}", tag=f"pab{i}")
                for i in range(4)  # index = 2*b + jc
            ]
            for b in range(BLOC):
                for it in range(2):
                    p = ps[2 * b + it]
                    mx = smx.tile([128, 1], F32, tag="mx", bufs=2)
                    nc.vector.reduce_max(
                        out=mx[:], in_=p[:], axis=mybir.AxisListType.X
                    )
                    negm = smx.tile([128, 1], F32, tag="negm", bufs=2)
                    nc.vector.tensor_scalar_mul(negm[:], mx[:], -1.0 / SC_SCALE)
                    pexp = smx.tile([128, S], F32, tag="pexp", bufs=2)
                    sm = smx.tile([128, 1], F32, tag="sm", bufs=2)
                    nc.scalar.activation(
                        pexp[:], p[:], mybir.ActivationFunctionType.Exp,
                        bias=negm[:], scale=1.0 / SC_SCALE, accum_out=sm[:],
                    )
                    rin = smx.tile([128, 1], F32, tag="rin", bufs=2)
                    nc.vector.reciprocal(rin[:], sm[:])
                    attn_b = smx.tile([128, S], BF, tag="attn", bufs=2)
                    nc.vector.tensor_scalar_mul(attn_b[:], pexp[:], rin[:])
                    for jc in range(2):
                        nc.tensor.matmul(
                            pab[2 * b + jc][:],
                            attn_b[:, jc * 128 : (jc + 1) * 128],
                            ones_s[:],
                            start=(it == 0), stop=(it == 1),
                            skip_group_check=True,
                        )
            for b in range(BLOC):
                for jc in range(2):
                    nc.vector.tensor_copy(
                        a_bar2[:, 2 * b + jc, b : b + 1], pab[2 * b + jc][:]
                    )

    # ------------------ phase 5: z1 = Gt^T abar, then bf16 MLP ------------
    with (
        tc.tile_pool(name="mlph", bufs=1) as mlph,
        tc.tile_pool(name="psum_m", bufs=1, space="PSUM") as psum_m,
    ):
        for ot in range(4):
            pm = psum_m.tile([128, BLOC], F32, tag="pm1", bufs=2)
            for tcn in range(4):
                nc.tensor.matmul(
                    pm[:],
                    gt_sb[:, tcn, ot * 128 : (ot + 1) * 128],
                    a_bar2[:, tcn, :],
                    start=(tcn == 0), stop=(tcn == 3),
                )
            nc.scalar.activation(
                h1T[:, ot, :], pm[:], mybir.ActivationFunctionType.Sigmoid,
                scale=1.0 / (XASCALE * WF_SCALE),
            )

        for ot in range(4):
            pm = psum_m.tile([128, BLOC], F32, tag="pm2", bufs=2)
            for oc in range(5):
                nc.tensor.matmul(
                    pm[:],
                    w2_t[:, oc, ot * 128 : (ot + 1) * 128],
                    h1T[:, oc, :],
                    start=(oc == 0), stop=(oc == 4),
                )
            nc.scalar.activation(
                h2T[:, ot, :], pm[:], mybir.ActivationFunctionType.Sigmoid
            )

        pm3 = psum_m.tile([C, BLOC], F32, tag="pm3")
        for oc in range(5):
            nc.tensor.matmul(
                pm3[:],
                w3_t[:, oc, :],
                h2T[:, oc, :],
                start=(oc == 0), stop=(oc == 4),
            )
        out_sb = mlph.tile([C, BLOC], F32)
        nc.vector.tensor_copy(out_sb[:], pm3[:])
        nc.sync.dma_start(out_d[:], out_sb[:])
    mlpw.release()


# ---------------------------------------------------------------------------
# Host-side packing
# ---------------------------------------------------------------------------
def _pack_m8(Wq, bq, Wk, bk):
    """M = Wq'^T Wk' / sqrt(D), where W' carries its bias in column d=3800.
    scores = x' M x'^T reproduces q @ k.T / sqrt(D) exactly (the unit bias
    feature of x' supplies the bias cross terms). Scaled by SC_SCALE for
    e4m3 range, DoubleRow-interleaved to [ET, 128, PAIRS, 2, 128]:
    A[d2t, d1p, p, ko, d2p] = SC_SCALE * M[(2p+ko)*128+d1p, d2t*128+d2p]."""
    Wqp = np.zeros((D, DP), dtype=np.float32)
    Wqp[:, :D] = Wq
    Wqp[:, D] = bq
    Wkp = np.zeros((D, DP), dtype=np.float32)
    Wkp[:, :D] = Wk
    Wkp[:, D] = bk
    M = (Wqp.T @ Wkp) * np.float32(SC_SCALE / np.sqrt(np.float64(D)))
    A = M.reshape(PAIRS, 2, 128, ET, 128).transpose(3, 2, 0, 1, 4)
    return np.ascontiguousarray(A, dtype=F8NP)


def _pack_wf(W1, b1, Wv, bv):
    """Wf = W1 @ Wv (+ bias col): z1 = Wf @ (abar' x') / (XASCALE*WF_SCALE).
    Layout [128, KC, H]: A[dp, kc, o] = WF_SCALE * Wf[o, kc*128+dp];
    column d=3800 carries W1@bv + b1 (x' has the unit feature there)."""
    Wf = np.zeros((H, DP), dtype=np.float32)
    Wf[:, :D] = (W1.astype(np.float64) @ Wv.astype(np.float64)).astype(
        np.float32
    )
    Wf[:, D] = W1 @ bv + b1
    Wf *= np.float32(WF_SCALE)
    A = Wf.T.reshape(KC, 128, H).transpose(1, 0, 2)
    return np.ascontiguousarray(A, dtype=F8NP)


def _pack_x8(xc):
    """xc [BLOC, S, D] -> [128, PAIRS, 2, T] e4m3, bias row d=3800 = 1."""
    xt = np.zeros((DP, T), dtype=np.float32)
    xt[:D, :] = xc.reshape(T, D).T
    xt[D, :] = 1.0
    A = xt.reshape(PAIRS, 2, 128, T).transpose(2, 0, 1, 3)
    return np.ascontiguousarray(A, dtype=F8NP)


def _pack_w2(W2, b2):
    A = np.zeros((128, 5, H), dtype=np.float32)
    A[:, :4, :] = W2.T.reshape(4, 128, H).transpose(1, 0, 2)
    A[0, 4, :] = b2
    return np.ascontiguousarray(A)


def _pack_w3(W3, b3):
    A = np.zeros((128, 5, C), dtype=np.float32)
    A[:, :4, :] = W3.T.reshape(4, 128, C).transpose(1, 0, 2)
    A[0, 4, :] = b3
    return np.ascontiguousarray(A)


_NC_CACHE = {}


def _get_nc():
    if "nc" not in _NC_CACHE:
        _NC_CACHE["nc"] = build_kernel()
    return _NC_CACHE["nc"]


def kernel(x, Wk, bk, Wq, bq, Wv, bv, W1, b1, W2, b2, W3, b3, _trace=False):
    x = np.asarray(x, dtype=np.float32)

    m8_p = _pack_m8(
        np.asarray(Wq, np.float32), np.asarray(bq, np.float32),
        np.asarray(Wk, np.float32), np.asarray(bk, np.float32),
    )
    wf_p = _pack_wf(
        np.asarray(W1, np.float32), np.asarray(b1, np.float32),
        np.asarray(Wv, np.float32), np.asarray(bv, np.float32),
    )
    w2_p = _pack_w2(np.asarray(W2, np.float32), np.asarray(b2, np.float32))
    w3_p = _pack_w3(np.asarray(W3, np.float32), np.asarray(b3, np.float32))
    e0b = np.zeros((128, BLOC), dtype=np.float32)
    e0b[0, :] = 1.0

    in_maps = []
    for c in range(N_CORES):
        xc = x[c * BLOC : (c + 1) * BLOC]
        in_maps.append(
            {
                "x8": _pack_x8(xc),
                "m8": m8_p,
                "wf": wf_p,
                "w2": w2_p,
                "w3": w3_p,
                "e0b": e0b,
            }
        )

    nc = _get_nc()
    _install_verbose_cc_hook()
    res = run_bass_kernel_spmd(nc, in_maps, list(range(N_CORES)), trace=_trace)
    out = np.zeros((B, C), dtype=np.float32)
    for c in range(N_CORES):
        out[c * BLOC : (c + 1) * BLOC] = res.results[c]["outT"].T
    if _trace:
        return out, res
    return out


# revision 24
# speedup vs baseline: 1.0014x; 1.0014x over previous
"""Trainium2 Bass kernel for AttentionMLP (nn_AttentionMLP_72997264163220).

Reference computation:
  k/q/v = x @ W{k,q,v}.T + b      (D=3800 -> D)
  scores = q @ k.T / sqrt(D); attn = softmax(scores, -1)
  attended = attn @ v; h = attended.mean(seq)
  h = sigmoid(h @ W1.T + b1); h = sigmoid(h @ W2.T + b2); out = h @ W3.T + b3

Algebraic simplifications (all weight-only, folded on the host):
  1. scores = x' M x'^T with M = Wq'^T Wk' / sqrt(D)  (bias via a unit
     feature at d=3800), so q/k are never materialized.
  2. The mean over the sequence commutes with attention and the linear
     V/W1 layers:  z1 = W1(Wv(abar @ x') + bv) + b1 = Wf @ (abar @ x')
     with Wf = W1 @ Wv (+ bias col).  On device this is evaluated as
       Gt = x' @ Wf^T   [tokens, H]   (reuses the x tiles already in
                                       SBUF for the scores matmuls)
       z1 = Gt^T @ abar [H, batch]
     so neither v, the attended tensor, xa, nor Wv/W1 are ever needed:
     only the 512xD folded Wf (fp8, ~2MB vs ~19MB for Wv+W1).

Sharding: data-parallel over batch. 16 batches -> 8 cores x 2 batches
(512 tokens per core). All weights replicated, host pre-transposed /
tiled / cast. Big matmuls in fp8 DoubleRow (fp32 PSUM accumulate);
softmax in fp32, MLP tail in bf16.

DMA queues: m8 (14.7MB, the big stream) rides the Sync queue alone;
x8 on the Vector queue; Wf/W2/W3/e0b on the Scalar queue. This keeps
the critical m8 stream free of head-of-line blocking (the previous
version put ~39MB on one queue and spent ~30us before compute ramped).

Device dataflow per core (SBUF partition dim always first; D padded to
3840 = 30*128 with a bias feature at d=3800):
  xT    [128, 30, 512] fp8  x^T (dp, kc, token); row d=3800 == 1
  per e-tile et in 30:  t1[et] = M8[et]^T x8   (DoubleRow pairs)
  scores[2b+it] psum [128,256] += t1_et_slice^T @ x8_slice  over et
  softmax rows (fp32, on ACT/DVE) -> attn bf16 [128(i), 256(j)]
  abar[b] = colsum_i(attn) * (XASCALE/S)  via matmul with a const vec
  Gt[tc]  = x8_tc^T @ Wf^T  (DoubleRow, abar-independent: runs during
            softmax) -> gt_sb [128(tok), 4, 512] bf16
  z1 = Gt^T abar -> sigmoid -> bf16 MLP (biases via unit rows/feature)
"""

import sys
import types

import numpy as np

if "/opt/trn_rl_repo" not in sys.path:
    sys.path.insert(0, "/opt/trn_rl_repo")


# ---------------------------------------------------------------------------
# NTFF profile hook shim (antenv.axon_hooks is absent in this image). Needed
# only when profiling (trace=True); harmless otherwise.
# ---------------------------------------------------------------------------
def _install_ntff_hook():
    try:
        import antenv  # noqa: F401

        if "antenv.axon_hooks" in sys.modules:
            return
        hooks_mod = types.ModuleType("antenv.axon_hooks")
        hooks_mod._hook = None

        def set_axon_ntff_profile_hook(h):
            hooks_mod._hook = h

        def get_axon_ntff_profile_hook():
            return hooks_mod._hook

        hooks_mod.set_axon_ntff_profile_hook = set_axon_ntff_profile_hook
        hooks_mod.get_axon_ntff_profile_hook = get_axon_ntff_profile_hook
        sys.modules["antenv.axon_hooks"] = hooks_mod
        import antenv as _a

        _a.axon_hooks = hooks_mod
        from trn_agent_boot.trn_boot import _ntff_profile_via_ctypes

        set_axon_ntff_profile_hook(
            _ntff_profile_via_ctypes("/opt/axon/libaxon_pjrt.so")
        )
    except Exception:
        pass


_install_ntff_hook()


def _install_verbose_cc_hook():
    """Wrap the PJRT->python compile callback so real tracebacks surface
    instead of an opaque 'CallFunctionObjArgs' error."""
    try:
        import traceback

        from concourse import bass2jax

        bass2jax.install_neuronx_cc_hook()
        import libneuronxla

        if getattr(libneuronxla, "_ant_verbose_wrap", False):
            return
        orig = libneuronxla.neuronx_cc

        def wrapped(*a, **k):
            try:
                return orig(*a, **k)
            except BaseException:
                traceback.print_exc()
                sys.stderr.flush()
                raise

        libneuronxla.neuronx_cc = wrapped
        libneuronxla._ant_verbose_wrap = True
        bass2jax.install_neuronx_cc_hook = lambda: None
    except Exception:
        pass


import bass_rust
import ml_dtypes
import concourse.bass as bass
import concourse.tile as tile
from concourse import mybir
from concourse.bass_utils import run_bass_kernel_spmd
from concourse.vector_clock import ScopedClock

BF16 = ml_dtypes.bfloat16

N_CORES = 8
B = 16  # batches total
S = 256  # seq len
D = 3800  # feature dim
H = 512  # hidden
C = 10  # classes

BLOC = B // N_CORES  # batches per core = 2
T = BLOC * S  # tokens per core = 512
DP = 3840  # D padded (+1 bias feature, up to 30*128)
KC = DP // 128  # 30 contraction chunks
ET = DP // 128  # 30 e-tiles of 128
PAIRS = KC // 2  # 15 DoubleRow chunk pairs
F32 = mybir.dt.float32
BF = mybir.dt.bfloat16
F8 = mybir.dt.float8e4
F8NP = mybir.dt.np(F8)  # ml_dtypes.float8_e4m3
# fp8 scale factors: weights are ~U(+-1/sqrt(3800)) which lands in e4m3's
# subnormal range, so weights are scaled up and the product scales are
# folded back out downstream (softmax scale / sigmoid scale).
SC_SCALE = 4096.0  # on M = Wq^T Wk / sqrt(D); scores' = 4096 * scores
XASCALE = 16.0  # on abar (via the ones vector)
WF_SCALE = 256.0  # on Wf = W1 @ Wv; z1' = XASCALE*WF_SCALE * z1


class SplitDrainTileContext(tile.TileContext):
    """This walrus build rejects >1 sync-wait on the tail Drain; split the
    global-clock waits across a chain of single-wait drain instructions."""

    MAXW = 1

    def _drain_and_barrier(self, tick_clock, wait_clock):
        # Distribute the final global-clock waits round-robin over all five
        # engines as single-wait NOPs (they execute in parallel, ~170ns
        # each) instead of a serial chain of single-wait Sync drains (~56
        # waits = ~9.5us of teardown). The barrier below joins the engines.
        nc = self.nc
        drain_inst = nc.sync.drain()
        wait_clock.add_sem_waits(
            drain_inst.ins, ScopedClock({None: tick_clock.global_clock})
        )
        si = drain_inst.ins.sync_info
        if si is not None and si.on_wait and len(si.on_wait) > self.MAXW:
            waits = list(si.on_wait)
            si.on_wait = waits[: self.MAXW]
            rest = waits[self.MAXW :]
            engines = [nc.tensor, nc.vector, nc.scalar, nc.gpsimd, nc.sync]
            for i, w in enumerate(rest):
                nop = engines[i % len(engines)].nop()
                nop.ins.sync_info = bass_rust.SyncInfo(
                    on_wait=[w], on_update=[]
                )
        nc.all_engine_barrier()
        assert self.sems is not None
        popped = nc._tile_sem_poison_stack.pop()
        assert popped is self._sem_poison
        nc.clear_and_free_semaphores(list(self.sems.allocated().values()))
        nc.all_engine_barrier()


def _fix_excess_waits(nc, aux_sem, maxw=1):
    """Walrus in this image rejects instructions with more than ~1 sync
    wait. Compute-engine instructions: hoist extra waits onto same-engine
    no-ops inserted just before (sequencers execute in order). DMACopy:
    its waits live in the DGE queue descriptor, so an SP-side chain waits
    on all the original conditions, bumps `aux_sem`, and the descriptor
    waits on aux_sem alone."""
    aux_count = 0
    for f in nc.m.functions:
        for bb in f.blocks:
            insts = bb.instructions
            if not any(
                i.sync_info and i.sync_info.on_wait
                and len(i.sync_info.on_wait) > maxw
                for i in insts
            ):
                continue
            out = []
            for ins in insts:
                si = ins.sync_info
                nw = len(si.on_wait) if si and si.on_wait else 0
                if nw > maxw:
                    waits = list(si.on_wait)
                    if isinstance(ins, mybir.InstDMACopy):
                        for j, w in enumerate(waits):
                            nop = mybir.InstNoOp(name=f"{ins.name}-dw{j}")
                            nop.engine = mybir.EngineType.SP
                            nop.sync_info = bass_rust.SyncInfo(
                                on_wait=[w], on_update=[]
                            )
                            out.append(nop)
                        aux_count += 1
                        inc = mybir.InstNoOp(name=f"{ins.name}-dinc")
                        inc.engine = mybir.EngineType.SP
                        inc.sync_info = bass_rust.SyncInfo(
                            on_wait=[],
                            on_update=[
                                bass_rust.SyncUpdate(
                                    sync_type="semaphore",
                                    id=aux_sem.num,
                                    ant_name=aux_sem.name,
                                    update_mode="sem-add-imm",
                                    update_value=1,
                                    update_reg=None,
                                )
                            ],
                        )
                        out.append(inc)
                        si.on_wait = [
                            bass_rust.SyncWait(
                                sync_type="semaphore",
                                id=aux_sem.num,
                                ant_name=aux_sem.name,
                                wait_mode="sem-ge-imm",
                                wait_value=aux_count,
                                wait_reg=None,
                            )
                        ]
                    else:
                        keep = waits[-maxw:]
                        rest = waits[:-maxw]
                        for j, w in enumerate(rest):
                            nop = mybir.InstNoOp(name=f"{ins.name}-xw{j}")
                            nop.engine = ins.engine
                            nop.sync_info = bass_rust.SyncInfo(
                                on_wait=[w], on_update=[]
                            )
                            out.append(nop)
                        si.on_wait = keep
                out.append(ins)
            bb.instructions = out
    if aux_count:
        # reset aux sem at the very end so a re-executed NEFF starts clean
        f = nc.m.functions[0]
        bb = list(f.blocks)[-1]
        rst = mybir.InstNoOp(name="auxwait-reset")
        rst.engine = mybir.EngineType.SP
        rst.sync_info = bass_rust.SyncInfo(
            on_wait=[],
            on_update=[
                bass_rust.SyncUpdate(
                    sync_type="semaphore",
                    id=aux_sem.num,
                    ant_name=aux_sem.name,
                    update_mode="sem-sub-imm",
                    update_value=aux_count,
                    update_reg=None,
                )
            ],
        )
        il = bb.instructions
        il.append(rst)
        bb.instructions = il


def build_kernel() -> bass.Bass:
    nc = bass.Bass()

    x_d = nc.declare_dram_parameter("x8", [128, PAIRS, 2, T], F8, isOutput=False)
    m8_d = nc.declare_dram_parameter("m8", [ET, 128, PAIRS, 2, 128], F8,
                                     isOutput=False)
    wf_d = nc.declare_dram_parameter("wf", [128, KC, H], F8, isOutput=False)
    # W2/W3 stay fp32: the batch-independent (constant) component of the
    # logits flows through them with no sigmoid small-signal suppression,
    # so bf16 weight error here hits the output directly (~2e-3).
    w2_d = nc.declare_dram_parameter("w2", [128, 5, H], F32, isOutput=False)
    w3_d = nc.declare_dram_parameter("w3", [128, 5, C], F32, isOutput=False)
    e0b_d = nc.declare_dram_parameter("e0b", [128, BLOC], F32, isOutput=False)
    out_d = nc.declare_dram_parameter("outT", [C, BLOC], F32, isOutput=True)

    aux_sem = nc.alloc_semaphore("auxwait")
    with SplitDrainTileContext(nc) as tc:
        with tc.tile_pool(name="persist", bufs=1) as persist:
            _emit(nc, tc, persist, x_d, m8_d, wf_d, w2_d, w3_d, e0b_d, out_d)
    _fix_excess_waits(nc, aux_sem)
    return nc


def _emit(nc, tc, persist, x_d, m8_d, wf_d, w2_d, w3_d, e0b_d, out_d):
    # ------------------ persistent tiles ------------------
    # x8 in 4 group tiles (4+4+4+3 DoubleRow pairs). Grouping matters two
    # ways: Tile dependencies are whole-tile (early matmuls only wait on
    # their own group's DMA), and every DMA costs ~600ns on the shared
    # HWDGE descriptor generator, so per-pair DMAs would make the startup
    # issue-bound (~1.2us/pair was the old kernel's 30us head).
    GRP = [4, 4, 4, 3]
    g0 = [0, 4, 8, 12]
    x8g = [persist.tile([128, GRP[g], 2, T], F8, name=f"x8g{g}", tag=f"x8g{g}")
           for g in range(4)]

    def x8p(p, ts=slice(None)):
        return x8g[p // 4][:, p % 4, :, ts]
    ones_s = persist.tile([128, 1], BF)
    nc.vector.memset(ones_s[:], XASCALE / S)
    a_bar2 = persist.tile([128, 4, BLOC], BF)
    nc.vector.memset(a_bar2[:], 0.0)
    gt_sb = persist.tile([128, 4, H], BF)  # Gt = x' Wf^T  (tok, tc, o)
    # t1 = (M8^T x8): fp8, [d2 within tile, d2-tile, token]
    t1_sb = persist.tile([128, KC, T], F8)
    # h1/h2 split per 128-chunk so z2/z3 matmuls start as soon as their
    # own chunk's sigmoid lands (Tile dependencies are whole-tile)
    h1c = [persist.tile([128, BLOC], F32, name=f"h1c{i}", tag=f"h1c{i}")
           for i in range(5)]
    h2c = [persist.tile([128, BLOC], F32, name=f"h2c{i}", tag=f"h2c{i}")
           for i in range(5)]

    # MLP weights: tiles up-front, DMAs issued a few iterations into
    # phase 1 so they overlap compute instead of the critical startup
    mlpw = tc.alloc_tile_pool(name="mlpw", bufs=1)
    wf_t = mlpw.tile([128, KC, H], F8)
    w2_t = mlpw.tile([128, 5, H], F32)
    w3_t = mlpw.tile([128, 5, C], F32)

    # ---- phase 1a: t1 = M^T x  (scores = x M x^T = t1^T x, M = Wq^T Wk) ----
    DR = mybir.MatmulPerfMode.DoubleRow
    with tc.tile_pool(name="psum_sc", bufs=1, space="PSUM") as psum_sc:
        ps = [
            psum_sc.tile([128, S], F32, name=f"scores{i}", tag=f"scores{i}")
            for i in range(4)  # index = 2*b + it
        ]
        with (
            tc.tile_pool(name="mpool", bufs=1) as mpool,
            tc.tile_pool(name="psum_kq", bufs=1, space="PSUM") as psum_kq,
        ):
            for d2t in range(ET):
                m_t = mpool.tile([128, PAIRS, 2, 128], F8, tag="m8", bufs=3)
                nc.sync.dma_start(m_t[:], m8_d[d2t])
                if d2t == 0:
                    # x8 rides the Activation queue in parallel with the
                    # m8 stream on Sync.
                    for g in range(4):
                        nc.scalar.dma_start(x8g[g][:], x_d[:, g0[g] : g0[g] + GRP[g]])
                if 2 <= d2t < 12 and d2t % 2 == 0:
                    # spread the Wf load over phase-1 iterations, 6 chunks
                    # per DMA (issue cost dominates smaller transfers)
                    kc0 = 3 * (d2t - 2)
                    nc.scalar.dma_start(wf_t[:, kc0 : kc0 + 6, :],
                                        wf_d[:, kc0 : kc0 + 6, :])
                if d2t == 12:
                    nc.scalar.dma_start(w2_t[:], w2_d[:])
                    nc.scalar.dma_start(w3_t[:], w3_d[:])
                    nc.scalar.dma_start(h1c[4][:], e0b_d[:])
                    nc.scalar.dma_start(h2c[4][:], e0b_d[:])

                pt = psum_kq.tile([128, T], F32, tag="pt", bufs=2)
                for p in range(PAIRS):
                    nc.tensor.matmul(
                        pt[:], m_t[:, p], x8p(p),
                        start=(p == 0), stop=(p == PAIRS - 1),
                        perf_mode=DR,
                    )
                nc.vector.tensor_copy(t1_sb[:, d2t, :], pt[:])

            # ---- phase 1b: scores'[i, j] = sum_d2 t1[d2, i] x8[d2, j] ----
            for b in range(BLOC):
                for it in range(2):
                    i0 = b * S + it * 128
                    for p in range(PAIRS):
                        nc.tensor.matmul(
                            ps[2 * b + it][:],
                            t1_sb[:, 2 * p : 2 * p + 2, i0 : i0 + 128],
                            x8p(p, slice(b * S, (b + 1) * S)),
                            start=(p == 0), stop=(p == PAIRS - 1),
                            perf_mode=DR,
                        )

        # ---- phase G: Gt = x' Wf^T (abar-independent; overlaps softmax) ----
        with tc.tile_pool(name="psum_g", bufs=1, space="PSUM") as psum_g:
            for tcn in range(4):
                pg = psum_g.tile([128, H], F32, tag="pg", bufs=2)
                for p in range(PAIRS):
                    nc.tensor.matmul(
                        pg[:],
                        x8p(p, slice(tcn * 128, (tcn + 1) * 128)),
                        wf_t[:, 2 * p : 2 * p + 2, :],
                        start=(p == 0), stop=(p == PAIRS - 1),
                        perf_mode=DR,
                    )
                nc.vector.tensor_copy(gt_sb[:, tcn, :], pg[:])

        # ------------- phase 2: softmax + abar (column means) -------------
        with (
            tc.tile_pool(name="smx", bufs=1) as smx,
            tc.tile_pool(name="psum_ab", bufs=1, space="PSUM") as psum_ab,
        ):
            pab = [
                psum_ab.tile([128, 1], F32, name=f"pab{i}", tag=f"pab{i}")
                for i in range(4)  # index = 2*b + jc
            ]
            for b in range(BLOC):
                for it in range(2):
                    p = ps[2 * b + it]
                    mx = smx.tile([128, 1], F32, tag="mx", bufs=2)
                    nc.vector.reduce_max(
                        out=mx[:], in_=p[:], axis=mybir.AxisListType.X
                    )
                    negm = smx.tile([128, 1], F32, tag="negm", bufs=2)
                    nc.vector.tensor_scalar_mul(negm[:], mx[:], -1.0 / SC_SCALE)
                    pexp = smx.tile([128, S], F32, tag="pexp", bufs=2)
                    sm = smx.tile([128, 1], F32, tag="sm", bufs=2)
                    nc.scalar.activation(
                        pexp[:], p[:], mybir.ActivationFunctionType.Exp,
                        bias=negm[:], scale=1.0 / SC_SCALE, accum_out=sm[:],
                    )
                    rin = smx.tile([128, 1], F32, tag="rin", bufs=2)
                    nc.vector.reciprocal(rin[:], sm[:])
                    attn_b = smx.tile([128, S], BF, tag="attn", bufs=2)
                    nc.vector.tensor_scalar_mul(attn_b[:], pexp[:], rin[:])
                    for jc in range(2):
                        nc.tensor.matmul(
                            pab[2 * b + jc][:],
                            attn_b[:, jc * 128 : (jc + 1) * 128],
                            ones_s[:],
                            start=(it == 0), stop=(it == 1),
                            skip_group_check=True,
                        )
            for b in range(BLOC):
                for jc in range(2):
                    nc.vector.tensor_copy(
                        a_bar2[:, 2 * b + jc, b : b + 1], pab[2 * b + jc][:]
                    )

    # ------------------ phase 5: z1 = Gt^T abar, then bf16 MLP ------------
    with (
        tc.tile_pool(name="mlph", bufs=1) as mlph,
        tc.tile_pool(name="psum_m", bufs=1, space="PSUM") as psum_m,
    ):
        for ot in range(4):
            pm = psum_m.tile([128, BLOC], F32, tag="pm1", bufs=3)
            for tcn in range(4):
                nc.tensor.matmul(
                    pm[:],
                    gt_sb[:, tcn, ot * 128 : (ot + 1) * 128],
                    a_bar2[:, tcn, :],
                    start=(tcn == 0), stop=(tcn == 3),
                )
            nc.scalar.activation(
                h1c[ot][:], pm[:], mybir.ActivationFunctionType.Sigmoid,
                scale=1.0 / (XASCALE * WF_SCALE),
            )

        for ot in range(4):
            pm = psum_m.tile([128, BLOC], F32, tag="pm2", bufs=3)
            for oc in range(5):
                nc.tensor.matmul(
                    pm[:],
                    w2_t[:, oc, ot * 128 : (ot + 1) * 128],
                    h1c[oc][:],
                    start=(oc == 0), stop=(oc == 4),
                )
            nc.scalar.activation(
                h2c[ot][:], pm[:], mybir.ActivationFunctionType.Sigmoid
            )

        pm3 = psum_m.tile([C, BLOC], F32, tag="pm3")
        for oc in range(5):
            nc.tensor.matmul(
                pm3[:],
                w3_t[:, oc, :],
                h2c[oc][:],
                start=(oc == 0), stop=(oc == 4),
            )
        out_sb = mlph.tile([C, BLOC], F32)
        nc.vector.tensor_copy(out_sb[:], pm3[:])
        nc.sync.dma_start(out_d[:], out_sb[:])
    mlpw.release()


# ---------------------------------------------------------------------------
# Host-side packing
# ---------------------------------------------------------------------------
def _pack_m8(Wq, bq, Wk, bk):
    """M = Wq'^T Wk' / sqrt(D), where W' carries its bias in column d=3800.
    scores = x' M x'^T reproduces q @ k.T / sqrt(D) exactly (the unit bias
    feature of x' supplies the bias cross terms). Scaled by SC_SCALE for
    e4m3 range, DoubleRow-interleaved to [ET, 128, PAIRS, 2, 128]:
    A[d2t, d1p, p, ko, d2p] = SC_SCALE * M[(2p+ko)*128+d1p, d2t*128+d2p]."""
    Wqp = np.zeros((D, DP), dtype=np.float32)
    Wqp[:, :D] = Wq
    Wqp[:, D] = bq
    Wkp = np.zeros((D, DP), dtype=np.float32)
    Wkp[:, :D] = Wk
    Wkp[:, D] = bk
    M = (Wqp.T @ Wkp) * np.float32(SC_SCALE / np.sqrt(np.float64(D)))
    A = M.reshape(PAIRS, 2, 128, ET, 128).transpose(3, 2, 0, 1, 4)
    return np.ascontiguousarray(A, dtype=F8NP)


def _pack_wf(W1, b1, Wv, bv):
    """Wf = W1 @ Wv (+ bias col): z1 = Wf @ (abar' x') / (XASCALE*WF_SCALE).
    Layout [128, KC, H]: A[dp, kc, o] = WF_SCALE * Wf[o, kc*128+dp];
    column d=3800 carries W1@bv + b1 (x' has the unit feature there)."""
    Wf = np.zeros((H, DP), dtype=np.float32)
    Wf[:, :D] = (W1.astype(np.float64) @ Wv.astype(np.float64)).astype(
        np.float32
    )
    Wf[:, D] = W1 @ bv + b1
    Wf *= np.float32(WF_SCALE)
    A = Wf.T.reshape(KC, 128, H).transpose(1, 0, 2)
    return np.ascontiguousarray(A, dtype=F8NP)


def _pack_x8(xc):
    """xc [BLOC, S, D] -> [128, PAIRS, 2, T] e4m3, bias row d=3800 = 1."""
    xt = np.zeros((DP, T), dtype=np.float32)
    xt[:D, :] = xc.reshape(T, D).T
    xt[D, :] = 1.0
    A = xt.reshape(PAIRS, 2, 128, T).transpose(2, 0, 1, 3)
    return np.ascontiguousarray(A, dtype=F8NP)


def _pack_w2(W2, b2):
    A = np.zeros((128, 5, H), dtype=np.float32)
    A[:, :4, :] = W2.T.reshape(4, 128, H).transpose(1, 0, 2)
    A[0, 4, :] = b2
    return np.ascontiguousarray(A)


def _pack_w3(W3, b3):
    A = np.zeros((128, 5, C), dtype=np.float32)
    A[:, :4, :] = W3.T.reshape(4, 128, C).transpose(1, 0, 2)
    A[0, 4, :] = b3
    return np.ascontiguousarray(A)


_NC_CACHE = {}


def _get_nc():
    if "nc" not in _NC_CACHE:
        _NC_CACHE["nc"] = build_kernel()
    return _NC_CACHE["nc"]


def kernel(x, Wk, bk, Wq, bq, Wv, bv, W1, b1, W2, b2, W3, b3, _trace=False):
    x = np.asarray(x, dtype=np.float32)

    m8_p = _pack_m8(
        np.asarray(Wq, np.float32), np.asarray(bq, np.float32),
        np.asarray(Wk, np.float32), np.asarray(bk, np.float32),
    )
    wf_p = _pack_wf(
        np.asarray(W1, np.float32), np.asarray(b1, np.float32),
        np.asarray(Wv, np.float32), np.asarray(bv, np.float32),
    )
    w2_p = _pack_w2(np.asarray(W2, np.float32), np.asarray(b2, np.float32))
    w3_p = _pack_w3(np.asarray(W3, np.float32), np.asarray(b3, np.float32))
    e0b = np.zeros((128, BLOC), dtype=np.float32)
    e0b[0, :] = 1.0

    in_maps = []
    for c in range(N_CORES):
        xc = x[c * BLOC : (c + 1) * BLOC]
        in_maps.append(
            {
                "x8": _pack_x8(xc),
                "m8": m8_p,
                "wf": wf_p,
                "w2": w2_p,
                "w3": w3_p,
                "e0b": e0b,
            }
        )

    nc = _get_nc()
    _install_verbose_cc_hook()
    res = run_bass_kernel_spmd(nc, in_maps, list(range(N_CORES)), trace=_trace)
    out = np.zeros((B, C), dtype=np.float32)
    for c in range(N_CORES):
        out[c * BLOC : (c + 1) * BLOC] = res.results[c]["outT"].T
    if _trace:
        return out, res
    return out


# revision 32
# speedup vs baseline: 1.0989x; 1.0973x over previous
"""Trainium2 Bass kernel for AttentionMLP (nn_AttentionMLP_72997264163220).

Reference computation:
  k/q/v = x @ W{k,q,v}.T + b      (D=3800 -> D)
  scores = q @ k.T / sqrt(D); attn = softmax(scores, -1)
  attended = attn @ v; h = attended.mean(seq)
  h = sigmoid(h @ W1.T + b1); h = sigmoid(h @ W2.T + b2); out = h @ W3.T + b3

Algebraic simplifications (all weight-only, folded on the host):
  1. scores = x' M x'^T with M = Wq'^T Wk' / sqrt(D)  (bias via a unit
     feature at d=3800), so q/k are never materialized.
  2. The mean over the sequence commutes with attention and the linear
     V/W1 layers:  z1 = W1(Wv(abar @ x') + bv) + b1 = Wf @ (abar @ x')
     with Wf = W1 @ Wv (+ bias col).  On device this is evaluated as
       Gt = x' @ Wf^T   [tokens, H]   (reuses the x tiles already in
                                       SBUF for the scores matmuls)
       z1 = Gt^T @ abar [H, batch]
     so neither v, the attended tensor, xa, nor Wv/W1 are ever needed:
     only the 512xD folded Wf (fp8, ~2MB vs ~19MB for Wv+W1).

Sharding: data-parallel over batch. 16 batches -> 8 cores x 2 batches
(512 tokens per core). All weights replicated, host pre-transposed /
tiled / cast. Big matmuls in fp8 DoubleRow (fp32 PSUM accumulate);
softmax in fp32, MLP tail in bf16.

DMA queues: m8 (14.7MB, the big stream) rides the Sync queue alone;
x8 on the Vector queue; Wf/W2/W3/e0b on the Scalar queue. This keeps
the critical m8 stream free of head-of-line blocking (the previous
version put ~39MB on one queue and spent ~30us before compute ramped).

Device dataflow per core (SBUF partition dim always first; D padded to
3840 = 30*128 with a bias feature at d=3800):
  xT    [128, 30, 512] fp8  x^T (dp, kc, token); row d=3800 == 1
  per e-tile et in 30:  t1[et] = M8[et]^T x8   (DoubleRow pairs)
  scores[2b+it] psum [128,256] += t1_et_slice^T @ x8_slice  over et
  softmax rows (fp32, on ACT/DVE) -> attn bf16 [128(i), 256(j)]
  abar[b] = colsum_i(attn) * (XASCALE/S)  via matmul with a const vec
  Gt[tc]  = x8_tc^T @ Wf^T  (DoubleRow, abar-independent: runs during
            softmax) -> gt_sb [128(tok), 4, 512] bf16
  z1 = Gt^T abar -> sigmoid -> bf16 MLP (biases via unit rows/feature)
"""

import sys
import types

import numpy as np

if "/opt/trn_rl_repo" not in sys.path:
    sys.path.insert(0, "/opt/trn_rl_repo")


# ---------------------------------------------------------------------------
# NTFF profile hook shim (antenv.axon_hooks is absent in this image). Needed
# only when profiling (trace=True); harmless otherwise.
# ---------------------------------------------------------------------------
def _install_ntff_hook():
    try:
        import antenv  # noqa: F401

        if "antenv.axon_hooks" in sys.modules:
            return
        hooks_mod = types.ModuleType("antenv.axon_hooks")
        hooks_mod._hook = None

        def set_axon_ntff_profile_hook(h):
            hooks_mod._hook = h

        def get_axon_ntff_profile_hook():
            return hooks_mod._hook

        hooks_mod.set_axon_ntff_profile_hook = set_axon_ntff_profile_hook
        hooks_mod.get_axon_ntff_profile_hook = get_axon_ntff_profile_hook
        sys.modules["antenv.axon_hooks"] = hooks_mod
        import antenv as _a

        _a.axon_hooks = hooks_mod
        from trn_agent_boot.trn_boot import _ntff_profile_via_ctypes

        set_axon_ntff_profile_hook(
            _ntff_profile_via_ctypes("/opt/axon/libaxon_pjrt.so")
        )
    except Exception:
        pass


_install_ntff_hook()


def _install_verbose_cc_hook():
    """Wrap the PJRT->python compile callback so real tracebacks surface
    instead of an opaque 'CallFunctionObjArgs' error."""
    try:
        import traceback

        from concourse import bass2jax

        bass2jax.install_neuronx_cc_hook()
        import libneuronxla

        if getattr(libneuronxla, "_ant_verbose_wrap", False):
            return
        orig = libneuronxla.neuronx_cc

        def wrapped(*a, **k):
            try:
                return orig(*a, **k)
            except BaseException:
                traceback.print_exc()
                sys.stderr.flush()
                raise

        libneuronxla.neuronx_cc = wrapped
        libneuronxla._ant_verbose_wrap = True
        bass2jax.install_neuronx_cc_hook = lambda: None
    except Exception:
        pass


import bass_rust
import ml_dtypes
import concourse.bass as bass
import concourse.tile as tile
from concourse import mybir
from concourse.bass_utils import run_bass_kernel_spmd
from concourse.vector_clock import ScopedClock

BF16 = ml_dtypes.bfloat16

N_CORES = 8
B = 16  # batches total
S = 256  # seq len
D = 3800  # feature dim
H = 512  # hidden
C = 10  # classes

BLOC = B // N_CORES  # batches per core = 2
T = BLOC * S  # tokens per core = 512
DP = 3840  # D padded (+1 bias feature, up to 30*128)
KC = DP // 128  # 30 contraction chunks
ET = DP // 128  # 30 e-tiles of 128
PAIRS = KC // 2  # 15 DoubleRow chunk pairs
F32 = mybir.dt.float32
BF = mybir.dt.bfloat16
F8 = mybir.dt.float8e4
F8NP = mybir.dt.np(F8)  # ml_dtypes.float8_e4m3
# fp8 scale factors: weights are ~U(+-1/sqrt(3800)) which lands in e4m3's
# subnormal range, so weights are scaled up and the product scales are
# folded back out downstream (softmax scale / sigmoid scale).
SC_SCALE = 4096.0  # on M = Wq^T Wk / sqrt(D); scores' = 4096 * scores
XASCALE = 16.0  # on abar (via the ones vector)
WF_SCALE = 256.0  # on Wf = W1 @ Wv; z1' = XASCALE*WF_SCALE * z1


class SplitDrainTileContext(tile.TileContext):
    """This walrus build rejects >1 sync-wait on the tail Drain; split the
    global-clock waits across a chain of single-wait drain instructions."""

    MAXW = 1

    def _drain_and_barrier(self, tick_clock, wait_clock):
        # Distribute the final global-clock waits round-robin over all five
        # engines as single-wait NOPs (they execute in parallel, ~170ns
        # each) instead of a serial chain of single-wait Sync drains (~56
        # waits = ~9.5us of teardown). The barrier below joins the engines.
        nc = self.nc
        drain_inst = nc.sync.drain()
        wait_clock.add_sem_waits(
            drain_inst.ins, ScopedClock({None: tick_clock.global_clock})
        )
        si = drain_inst.ins.sync_info
        if si is not None and si.on_wait and len(si.on_wait) > self.MAXW:
            waits = list(si.on_wait)
            si.on_wait = waits[: self.MAXW]
            rest = waits[self.MAXW :]
            engines = [nc.tensor, nc.vector, nc.scalar, nc.gpsimd, nc.sync]
            for i, w in enumerate(rest):
                nop = engines[i % len(engines)].nop()
                nop.ins.sync_info = bass_rust.SyncInfo(
                    on_wait=[w], on_update=[]
                )
        nc.all_engine_barrier()
        assert self.sems is not None
        popped = nc._tile_sem_poison_stack.pop()
        assert popped is self._sem_poison
        nc.clear_and_free_semaphores(list(self.sems.allocated().values()))
        nc.all_engine_barrier()


def _fix_excess_waits(nc, aux_sem, maxw=1):
    """Walrus in this image rejects instructions with more than ~1 sync
    wait. Compute-engine instructions: hoist extra waits onto same-engine
    no-ops inserted just before (sequencers execute in order). DMACopy:
    its waits live in the DGE queue descriptor, so an SP-side chain waits
    on all the original conditions, bumps `aux_sem`, and the descriptor
    waits on aux_sem alone."""
    aux_count = 0
    for f in nc.m.functions:
        for bb in f.blocks:
            insts = bb.instructions
            if not any(
                i.sync_info and i.sync_info.on_wait
                and len(i.sync_info.on_wait) > maxw
                for i in insts
            ):
                continue
            out = []
            for ins in insts:
                si = ins.sync_info
                nw = len(si.on_wait) if si and si.on_wait else 0
                if nw > maxw:
                    waits = list(si.on_wait)
                    if isinstance(ins, mybir.InstDMACopy):
                        for j, w in enumerate(waits):
                            nop = mybir.InstNoOp(name=f"{ins.name}-dw{j}")
                            nop.engine = mybir.EngineType.SP
                            nop.sync_info = bass_rust.SyncInfo(
                                on_wait=[w], on_update=[]
                            )
                            out.append(nop)
                        aux_count += 1
                        inc = mybir.InstNoOp(name=f"{ins.name}-dinc")
                        inc.engine = mybir.EngineType.SP
                        inc.sync_info = bass_rust.SyncInfo(
                            on_wait=[],
                            on_update=[
                                bass_rust.SyncUpdate(
                                    sync_type="semaphore",
                                    id=aux_sem.num,
                                    ant_name=aux_sem.name,
                                    update_mode="sem-add-imm",
                                    update_value=1,
                                    update_reg=None,
                                )
                            ],
                        )
                        out.append(inc)
                        si.on_wait = [
                            bass_rust.SyncWait(
                                sync_type="semaphore",
                                id=aux_sem.num,
                                ant_name=aux_sem.name,
                                wait_mode="sem-ge-imm",
                                wait_value=aux_count,
                                wait_reg=None,
                            )
                        ]
                    else:
                        keep = waits[-maxw:]
                        rest = waits[:-maxw]
                        for j, w in enumerate(rest):
                            nop = mybir.InstNoOp(name=f"{ins.name}-xw{j}")
                            nop.engine = ins.engine
                            nop.sync_info = bass_rust.SyncInfo(
                                on_wait=[w], on_update=[]
                            )
                            out.append(nop)
                        si.on_wait = keep
                out.append(ins)
            bb.instructions = out
    if aux_count:
        # reset aux sem at the very end so a re-executed NEFF starts clean
        f = nc.m.functions[0]
        bb = list(f.blocks)[-1]
        rst = mybir.InstNoOp(name="auxwait-reset")
        rst.engine = mybir.EngineType.SP
        rst.sync_info = bass_rust.SyncInfo(
            on_wait=[],
            on_update=[
                bass_rust.SyncUpdate(
                    sync_type="semaphore",
                    id=aux_sem.num,
                    ant_name=aux_sem.name,
                    update_mode="sem-sub-imm",
                    update_value=aux_count,
                    update_reg=None,
                )
            ],
        )
        il = bb.instructions
        il.append(rst)
        bb.instructions = il


def build_kernel() -> bass.Bass:
    nc = bass.Bass()

    x_d = nc.declare_dram_parameter("x8", [128, PAIRS, 2, T], F8, isOutput=False)
    m8_d = nc.declare_dram_parameter("m8", [ET, 128, PAIRS, 2, 128], F8,
                                     isOutput=False)
    wf_d = nc.declare_dram_parameter("wf", [128, KC, H], F8, isOutput=False)
    # MLP tail is centered: h = 0.5 + 0.5*tanh(z/2), with the weight-only
    # constants (0.5*W*1+b terms, and tanh(c2/2) for stage 3) folded on the
    # host in fp32 and injected through the activation-bias port / a DVE
    # add. The matmuls then only carry the tiny tanh deltas, so W2/W3 can
    # be bf16 (fp32 matmuls run LOW/HIGH double-pass at ~430ns each; this
    # was ~11us of tail).
    w2_d = nc.declare_dram_parameter("w2", [128, 4, H], BF, isOutput=False)
    w3_d = nc.declare_dram_parameter("w3", [128, 4, C], BF, isOutput=False)
    # cb: cols 0-3 = 0.5*c2 chunks, cols 4-7 = -tanh(c2/2) chunks,
    #     col 8 partitions 0..C-1 = c3' = 0.5*W3@(1+tanh(c2/2)) + b3
    cb_d = nc.declare_dram_parameter("cb", [128, 9], F32, isOutput=False)
    out_d = nc.declare_dram_parameter("outT", [C, BLOC], F32, isOutput=True)

    aux_sem = nc.alloc_semaphore("auxwait")
    with SplitDrainTileContext(nc) as tc:
        with tc.tile_pool(name="persist", bufs=1) as persist:
            _emit(nc, tc, persist, x_d, m8_d, wf_d, w2_d, w3_d, cb_d, out_d)
    _fix_excess_waits(nc, aux_sem)
    return nc


def _emit(nc, tc, persist, x_d, m8_d, wf_d, w2_d, w3_d, cb_d, out_d):
    # ------------------ persistent tiles ------------------
    # x8 in 4 group tiles (4+4+4+3 DoubleRow pairs). Grouping matters two
    # ways: Tile dependencies are whole-tile (early matmuls only wait on
    # their own group's DMA), and every DMA costs ~600ns on the shared
    # HWDGE descriptor generator, so per-pair DMAs would make the startup
    # issue-bound (~1.2us/pair was the old kernel's 30us head).
    GRP = [4, 4, 4, 3]
    g0 = [0, 4, 8, 12]
    x8g = [persist.tile([128, GRP[g], 2, T], F8, name=f"x8g{g}", tag=f"x8g{g}")
           for g in range(4)]

    def x8p(p, ts=slice(None)):
        return x8g[p // 4][:, p % 4, :, ts]
    ones_s = persist.tile([128, 1], BF)
    nc.vector.memset(ones_s[:], XASCALE / S)
    a_bar2 = persist.tile([128, 4, BLOC], BF)
    nc.vector.memset(a_bar2[:], 0.0)
    gt_sb = persist.tile([128, 4, H], BF)  # Gt = x' Wf^T  (tok, tc, o)
    # t1 = (M8^T x8): fp8, [d2 within tile, d2-tile, token]
    t1_sb = persist.tile([128, KC, T], F8)
    # tanh deltas, split per 128-chunk so downstream matmuls start as soon
    # as their own chunk's activation lands (Tile dependencies are
    # whole-tile)
    d1c = [persist.tile([128, BLOC], BF, name=f"d1c{i}", tag=f"d1c{i}")
           for i in range(4)]
    d2r = [persist.tile([128, BLOC], F32, name=f"d2r{i}", tag=f"d2r{i}")
           for i in range(4)]
    dl2c = [persist.tile([128, BLOC], BF, name=f"dl2c{i}", tag=f"dl2c{i}")
            for i in range(4)]
    warm = persist.tile([128, 1], BF)

    # MLP weights: tiles up-front, DMAs issued a few iterations into
    # phase 1 so they overlap compute instead of the critical startup
    mlpw = tc.alloc_tile_pool(name="mlpw", bufs=1)
    wf_t = mlpw.tile([128, KC, H], F8)
    w2_t = mlpw.tile([128, 4, H], BF)
    w3_t = mlpw.tile([128, 4, C], BF)
    cb_t = mlpw.tile([128, 9], F32)

    # ---- phase 1a: t1 = M^T x  (scores = x M x^T = t1^T x, M = Wq^T Wk) ----
    DR = mybir.MatmulPerfMode.DoubleRow
    with tc.tile_pool(name="psum_sc", bufs=1, space="PSUM") as psum_sc:
        ps = [
            psum_sc.tile([128, S], F32, name=f"scores{i}", tag=f"scores{i}")
            for i in range(4)  # index = 2*b + it
        ]
        with (
            tc.tile_pool(name="mpool", bufs=1) as mpool,
            tc.tile_pool(name="psum_kq", bufs=1, space="PSUM") as psum_kq,
        ):
            for d2t in range(ET):
                m_t = mpool.tile([128, PAIRS, 2, 128], F8, tag="m8", bufs=3)
                nc.sync.dma_start(m_t[:], m8_d[d2t])
                if d2t == 0:
                    # x8 rides the Activation queue in parallel with the
                    # m8 stream on Sync.
                    for g in range(4):
                        nc.scalar.dma_start(x8g[g][:], x_d[:, g0[g] : g0[g] + GRP[g]])
                if 2 <= d2t < 12 and d2t % 2 == 0:
                    # spread the Wf load over phase-1 iterations, 6 chunks
                    # per DMA (issue cost dominates smaller transfers)
                    kc0 = 3 * (d2t - 2)
                    nc.scalar.dma_start(wf_t[:, kc0 : kc0 + 6, :],
                                        wf_d[:, kc0 : kc0 + 6, :])
                if d2t == 1:
                    # warm the ACT tanh table while the engine is idle, so
                    # the tail's first real tanh skips the ~1.3us load
                    nc.scalar.activation(
                        warm[:], ones_s[:], mybir.ActivationFunctionType.Tanh
                    )
                if d2t == 12:
                    nc.scalar.dma_start(w2_t[:], w2_d[:])
                    nc.scalar.dma_start(w3_t[:], w3_d[:])
                    nc.scalar.dma_start(cb_t[:], cb_d[:])

                pt = psum_kq.tile([128, T], F32, tag="pt", bufs=2)
                for p in range(PAIRS):
                    nc.tensor.matmul(
                        pt[:], m_t[:, p], x8p(p),
                        start=(p == 0), stop=(p == PAIRS - 1),
                        perf_mode=DR,
                    )
                nc.vector.tensor_copy(t1_sb[:, d2t, :], pt[:])

            # ---- phase 1b: scores'[i, j] = sum_d2 t1[d2, i] x8[d2, j] ----
            for b in range(BLOC):
                for it in range(2):
                    i0 = b * S + it * 128
                    for p in range(PAIRS):
                        nc.tensor.matmul(
                            ps[2 * b + it][:],
                            t1_sb[:, 2 * p : 2 * p + 2, i0 : i0 + 128],
                            x8p(p, slice(b * S, (b + 1) * S)),
                            start=(p == 0), stop=(p == PAIRS - 1),
                            perf_mode=DR,
                        )

        # ---- phase G: Gt = x' Wf^T (abar-independent; overlaps softmax) ----
        with tc.tile_pool(name="psum_g", bufs=1, space="PSUM") as psum_g:
            for tcn in range(4):
                pg = psum_g.tile([128, H], F32, tag="pg", bufs=2)
                for p in range(PAIRS):
                    nc.tensor.matmul(
                        pg[:],
                        x8p(p, slice(tcn * 128, (tcn + 1) * 128)),
                        wf_t[:, 2 * p : 2 * p + 2, :],
                        start=(p == 0), stop=(p == PAIRS - 1),
                        perf_mode=DR,
                    )
                nc.vector.tensor_copy(gt_sb[:, tcn, :], pg[:])

        # ------------- phase 2: softmax + abar (column means) -------------
        with (
            tc.tile_pool(name="smx", bufs=1) as smx,
            tc.tile_pool(name="psum_ab", bufs=1, space="PSUM") as psum_ab,
        ):
            pab = [
                psum_ab.tile([128, 1], F32, name=f"pab{i}", tag=f"pab{i}")
                for i in range(4)  # index = 2*b + jc
            ]
            for b in range(BLOC):
                for it in range(2):
                    p = ps[2 * b + it]
                    mx = smx.tile([128, 1], F32, tag="mx", bufs=2)
                    nc.vector.reduce_max(
                        out=mx[:], in_=p[:], axis=mybir.AxisListType.X
                    )
                    negm = smx.tile([128, 1], F32, tag="negm", bufs=2)
                    nc.vector.tensor_scalar_mul(negm[:], mx[:], -1.0 / SC_SCALE)
                    pexp = smx.tile([128, S], F32, tag="pexp", bufs=2)
                    sm = smx.tile([128, 1], F32, tag="sm", bufs=2)
                    nc.scalar.activation(
                        pexp[:], p[:], mybir.ActivationFunctionType.Exp,
                        bias=negm[:], scale=1.0 / SC_SCALE, accum_out=sm[:],
                    )
                    rin = smx.tile([128, 1], F32, tag="rin", bufs=2)
                    nc.vector.reciprocal(rin[:], sm[:])
                    attn_b = smx.tile([128, S], BF, tag="attn", bufs=2)
                    nc.vector.tensor_scalar_mul(attn_b[:], pexp[:], rin[:])
                    for jc in range(2):
                        nc.tensor.matmul(
                            pab[2 * b + jc][:],
                            attn_b[:, jc * 128 : (jc + 1) * 128],
                            ones_s[:],
                            start=(it == 0), stop=(it == 1),
                            skip_group_check=True,
                        )
            for b in range(BLOC):
                for jc in range(2):
                    nc.vector.tensor_copy(
                        a_bar2[:, 2 * b + jc, b : b + 1], pab[2 * b + jc][:]
                    )

    # ------------------ phase 5: z1 = Gt^T abar, then bf16 MLP ------------
    with (
        tc.tile_pool(name="mlph", bufs=1) as mlph,
        tc.tile_pool(name="psum_m", bufs=1, space="PSUM") as psum_m,
    ):
        for ot in range(4):
            pm = psum_m.tile([128, BLOC], F32, tag="pm1", bufs=3)
            for tcn in range(4):
                nc.tensor.matmul(
                    pm[:],
                    gt_sb[:, tcn, ot * 128 : (ot + 1) * 128],
                    a_bar2[:, tcn, :],
                    start=(tcn == 0), stop=(tcn == 3),
                )
            # d1 = tanh(z1/2)
            nc.scalar.activation(
                d1c[ot][:], pm[:], mybir.ActivationFunctionType.Tanh,
                scale=0.5 / (XASCALE * WF_SCALE),
            )

        for ot in range(4):
            pm = psum_m.tile([128, BLOC], F32, tag="pm2", bufs=3)
            for oc in range(4):
                nc.tensor.matmul(
                    pm[:],
                    w2_t[:, oc, ot * 128 : (ot + 1) * 128],
                    d1c[oc][:],
                    start=(oc == 0), stop=(oc == 3),
                )
            # d2 = tanh(0.5*(W2h d1) + 0.5*c2); dl2 = d2 - tanh(c2/2) in
            # fp32 (bf16 first would cancel catastrophically)
            nc.scalar.activation(
                d2r[ot][:], pm[:], mybir.ActivationFunctionType.Tanh,
                scale=0.5, bias=cb_t[:, ot : ot + 1],
            )
            nc.vector.tensor_scalar_add(
                dl2c[ot][:], d2r[ot][:], cb_t[:, 4 + ot : 5 + ot]
            )

        pm3 = psum_m.tile([C, BLOC], F32, tag="pm3")
        for oc in range(4):
            nc.tensor.matmul(
                pm3[:],
                w3_t[:, oc, :],
                dl2c[oc][:],
                start=(oc == 0), stop=(oc == 3),
            )
        out_sb = mlph.tile([C, BLOC], F32)
        nc.vector.tensor_scalar_add(out_sb[:], pm3[:], cb_t[:C, 8:9])
        nc.sync.dma_start(out_d[:], out_sb[:])
    mlpw.release()


# ---------------------------------------------------------------------------
# Host-side packing
# ---------------------------------------------------------------------------
def _pack_m8(Wq, bq, Wk, bk):
    """M = Wq'^T Wk' / sqrt(D), where W' carries its bias in column d=3800.
    scores = x' M x'^T reproduces q @ k.T / sqrt(D) exactly (the unit bias
    feature of x' supplies the bias cross terms). Scaled by SC_SCALE for
    e4m3 range, DoubleRow-interleaved to [ET, 128, PAIRS, 2, 128]:
    A[d2t, d1p, p, ko, d2p] = SC_SCALE * M[(2p+ko)*128+d1p, d2t*128+d2p]."""
    Wqp = np.zeros((D, DP), dtype=np.float32)
    Wqp[:, :D] = Wq
    Wqp[:, D] = bq
    Wkp = np.zeros((D, DP), dtype=np.float32)
    Wkp[:, :D] = Wk
    Wkp[:, D] = bk
    M = (Wqp.T @ Wkp) * np.float32(SC_SCALE / np.sqrt(np.float64(D)))
    A = M.reshape(PAIRS, 2, 128, ET, 128).transpose(3, 2, 0, 1, 4)
    return np.ascontiguousarray(A, dtype=F8NP)


def _pack_wf(W1, b1, Wv, bv):
    """Wf = W1 @ Wv (+ bias col): z1 = Wf @ (abar' x') / (XASCALE*WF_SCALE).
    Layout [128, KC, H]: A[dp, kc, o] = WF_SCALE * Wf[o, kc*128+dp];
    column d=3800 carries W1@bv + b1 (x' has the unit feature there)."""
    Wf = np.zeros((H, DP), dtype=np.float32)
    Wf[:, :D] = (W1.astype(np.float64) @ Wv.astype(np.float64)).astype(
        np.float32
    )
    Wf[:, D] = W1 @ bv + b1
    Wf *= np.float32(WF_SCALE)
    A = Wf.T.reshape(KC, 128, H).transpose(1, 0, 2)
    return np.ascontiguousarray(A, dtype=F8NP)


def _pack_x8(xc):
    """xc [BLOC, S, D] -> [128, PAIRS, 2, T] e4m3, bias row d=3800 = 1."""
    xt = np.zeros((DP, T), dtype=np.float32)
    xt[:D, :] = xc.reshape(T, D).T
    xt[D, :] = 1.0
    A = xt.reshape(PAIRS, 2, 128, T).transpose(2, 0, 1, 3)
    return np.ascontiguousarray(A, dtype=F8NP)


def _pack_w2(W2):
    """0.5*W2 as [128, 4, H] bf16: A[hp, oc, o] = 0.5*W2[o, oc*128+hp]."""
    A = (0.5 * W2).T.reshape(4, 128, H).transpose(1, 0, 2)
    return np.ascontiguousarray(A, dtype=BF16)


def _pack_w3(W3):
    """0.5*W3 as [128, 4, C] bf16: A[hp, oc, o] = 0.5*W3[o, oc*128+hp]."""
    A = (0.5 * W3).T.reshape(4, 128, C).transpose(1, 0, 2)
    return np.ascontiguousarray(A, dtype=BF16)


def _pack_cb(W2, b2, W3, b3):
    """Centered-MLP constants (all fp32, exact): cols 0-3 = 0.5*c2 chunks
    (activation bias for stage-2 tanh), cols 4-7 = -tanh(c2/2) chunks
    (stage-3 centering), col 8 = c3' = 0.5*W3@(1+tanh(c2/2)) + b3."""
    c2 = 0.5 * W2.sum(1) + b2
    t2c = np.tanh(0.5 * c2)
    c3p = 0.5 * W3.sum(1) + b3 + 0.5 * (W3 @ t2c)
    A = np.zeros((128, 9), dtype=np.float32)
    A[:, 0:4] = (0.5 * c2).reshape(4, 128).T
    A[:, 4:8] = (-t2c).reshape(4, 128).T
    A[:C, 8] = c3p
    return np.ascontiguousarray(A)


_NC_CACHE = {}


def _get_nc():
    if "nc" not in _NC_CACHE:
        _NC_CACHE["nc"] = build_kernel()
    return _NC_CACHE["nc"]


def kernel(x, Wk, bk, Wq, bq, Wv, bv, W1, b1, W2, b2, W3, b3, _trace=False):
    x = np.asarray(x, dtype=np.float32)

    m8_p = _pack_m8(
        np.asarray(Wq, np.float32), np.asarray(bq, np.float32),
        np.asarray(Wk, np.float32), np.asarray(bk, np.float32),
    )
    wf_p = _pack_wf(
        np.asarray(W1, np.float32), np.asarray(b1, np.float32),
        np.asarray(Wv, np.float32), np.asarray(bv, np.float32),
    )
    W2f = np.asarray(W2, np.float32)
    W3f = np.asarray(W3, np.float32)
    w2_p = _pack_w2(W2f)
    w3_p = _pack_w3(W3f)
    cb_p = _pack_cb(W2f, np.asarray(b2, np.float32),
                    W3f, np.asarray(b3, np.float32))

    in_maps = []
    for c in range(N_CORES):
        xc = x[c * BLOC : (c + 1) * BLOC]
        in_maps.append(
            {
                "x8": _pack_x8(xc),
                "m8": m8_p,
                "wf": wf_p,
                "w2": w2_p,
                "w3": w3_p,
                "cb": cb_p,
            }
        )

    nc = _get_nc()
    _install_verbose_cc_hook()
    res = run_bass_kernel_spmd(nc, in_maps, list(range(N_CORES)), trace=_trace)
    out = np.zeros((B, C), dtype=np.float32)
    for c in range(N_CORES):
        out[c * BLOC : (c + 1) * BLOC] = res.results[c]["outT"].T
    if _trace:
        return out, res
    return out
